# revision 15
# baseline (speedup 1.0000x reference)
"""Trainium2 Bass kernel for nn_Encoder_7662221656403 (retrieval_knn).

Sharding: batch-parallel MLP (250 rows/core x 8 cores), negative-sharded
similarities (7500 negatives/core, all 2000 rows on every core).
Top-1000-of-60000 handled via a Newton threshold search on counts
(ACT Sign+accum passes over the sims) + corrected exp-sum formula:
    S_top1000 = e^{20 t} * (sum_j e^{20 relu(x_j - t)} - n + 1000)
exact to ~1e-4 relative once count(x>=t) is within ~100 of 1000.
"""
import sys
import os

sys.path.insert(0, '/opt/trn_rl_repo')

import numpy as np
import ml_dtypes

import concourse.bass as bass
import concourse.mybir as mybir
from concourse import bacc
from concourse.tile import TileContext
from concourse.bass_utils import run_bass_kernel_spmd
from concourse.masks import make_identity

f32 = mybir.dt.float32
f16 = mybir.dt.float16
bf16 = mybir.dt.bfloat16
i32 = mybir.dt.int32
AF = mybir.ActivationFunctionType
ALU = mybir.AluOpType

# ---------------- problem constants ----------------
N_CORES = 8
B = 2000
RB = B // N_CORES            # 250 rows per core
RH = RB // 2                 # 125 (partition tile size for rows)
NLEN = 10
NLAB = 10
EMB = 500
EMBP = 512
TEMB = 50
N_AID = 200000
PAD = N_AID - 1
H_CLIP = 845
H_ROWS = 846
SC = 20.0                    # 1 / TEMP
N_NEG = 60000
NEG_L = N_NEG // N_CORES     # 7500 per core
NEG_LP = 7552                # padded to 59*128
NEG_T = NEG_LP // 128        # 59 gather tiles
N_ALL = NEG_LP * N_CORES     # 60416 (incl pads; pads are sims=0 -> exp(0)=1)

# padded/permuted feature layout (see host_prep):
#   em block:  l*512 + d       (d in 0..499)  -> rows 0..5119
#   emh block: 5120 + l*50 + d -> rows 5120..5619 (pad to 5632)
#   other blk: 5632 + l*3 + k  (sin, cos, feat) -> 5632..5661
INP_P = 5760                 # 45 tiles of 128
INP_T = INP_P // 128
EMH0 = 5120
OTH0 = 5632
H1_P = 5632                  # 44 tiles
H1_T = H1_P // 128
H2_P = 2816                  # 22 tiles
H2_T = H2_P // 128
RT = 16                      # 16 r-tiles of 125 rows = 2000 global rows

SIGMA = 1.0 / np.sqrt(500.0)
T0 = 2.1280 * SIGMA          # ~0.09516 initial threshold guess
NSLOPE = N_NEG * float(np.exp(-0.5 * (T0 / SIGMA) ** 2)
                       / np.sqrt(2 * np.pi)) / SIGMA   # -dN/dt at T0

DEBUG_OUT = bool(int(os.environ.get("KERNEL_DEBUG_OUT", "0")))

_NC_CACHE = {}


def build_nc():
    nc = bacc.Bacc("TRN2", target_bir_lowering=False, debug=False,
                   num_devices=N_CORES)

    # ---------------- I/O ----------------
    xy = nc.dram_tensor("xy", [RB, 50], i32, kind="ExternalInput")
    negidx = nc.dram_tensor("negidx", [NEG_LP, 1], i32, kind="ExternalInput")
    emb_w = nc.dram_tensor("emb_w", [N_AID, EMB], f32, kind="ExternalInput")
    hour_tbl = nc.dram_tensor("hour_tbl", [H_ROWS, 52], f32,
                              kind="ExternalInput")
    lt_w = nc.dram_tensor("lt_w", [7, EMB], f32, kind="ExternalInput")
    w1p = nc.dram_tensor("w1p", [INP_P, H1_P], bf16, kind="ExternalInput")
    w2p = nc.dram_tensor("w2p", [H1_P, H2_P], bf16, kind="ExternalInput")
    w3p = nc.dram_tensor("w3p", [H2_P, EMBP], bf16, kind="ExternalInput")
    b1p = nc.dram_tensor("b1p", [H1_P, 1], f32, kind="ExternalInput")
    b2p = nc.dram_tensor("b2p", [H2_P, 1], f32, kind="ExternalInput")
    b3p = nc.dram_tensor("b3p", [EMBP, 1], f32, kind="ExternalInput")
    bn1 = nc.dram_tensor("bn1", [INP_P, 2], f32, kind="ExternalInput")
    bn2 = nc.dram_tensor("bn2", [H1_P, 2], f32, kind="ExternalInput")
    bn3 = nc.dram_tensor("bn3", [H2_P, 2], f32, kind="ExternalInput")

    out = nc.dram_tensor("out", [2, B], f32, kind="ExternalOutput")
    if DEBUG_OUT:
        dbg_q = nc.dram_tensor("dbg_q", [RB, EMBP], f32, kind="ExternalOutput")
        dbg_cpos = nc.dram_tensor("dbg_cpos", [B, 1], f32,
                                  kind="ExternalOutput")
        dbg_cnt = nc.dram_tensor("dbg_cnt", [B, 1], f32, kind="ExternalOutput")
        dbg_cnt1 = nc.dram_tensor("dbg_cnt1", [B, 1], f32,
                                  kind="ExternalOutput")
        dbg_t2 = nc.dram_tensor("dbg_t2", [B, 1], f32, kind="ExternalOutput")
        dbg_A = nc.dram_tensor("dbg_A", [B, 1], f32, kind="ExternalOutput")
        dbg_M = nc.dram_tensor("dbg_M", [B, 1], f32, kind="ExternalOutput")
        dbg_s1l = nc.dram_tensor("dbg_s1l", [INP_P, 2], f32,
                                 kind="ExternalOutput")
        dbg_s1g = nc.dram_tensor("dbg_s1g", [INP_P, 2], f32,
                                 kind="ExternalOutput")
        dbg_f0 = nc.dram_tensor("dbg_f0", [128, RB], f32,
                                kind="ExternalOutput")
        dbg_f44 = nc.dram_tensor("dbg_f44", [128, RB], f32,
                                 kind="ExternalOutput")
        dbg_h1 = nc.dram_tensor("dbg_h1", [128, RB], f32,
                                kind="ExternalOutput")

    RG = [list(range(N_CORES))]

    with TileContext(nc) as tc:
        with tc.tile_pool(name="persist", bufs=1) as pst, \
             tc.tile_pool(name="gath", bufs=4) as gp, \
             tc.tile_pool(name="work", bufs=3) as wp, \
             tc.tile_pool(name="small", bufs=1) as sp, \
             tc.tile_pool(name="mmps", bufs=6, space="PSUM") as mmps, \
             tc.tile_pool(name="trps", bufs=2, space="PSUM") as trps, \
             tc.tile_pool(name="dram", bufs=1, space="DRAM") as dram:

            identb = pst.tile([128, 128], bf16, tag="identb")
            make_identity(nc, identb[:])

            _consts = {}

            def constv(val, parts=128):
                if val not in _consts:
                    ct = pst.tile([128, 1], f32, tag=f"const{len(_consts)}")
                    nc.vector.memset(ct[:], float(val))
                    _consts[val] = ct
                return _consts[val][:parts, :1]

            def tr_psum(shape=(128, 128)):
                return trps.tile(list(shape), bf16, space="PSUM", tag="trp", name="trp")

            def logical_row_copy(dst_tile, row0, nrows, src, col0, ncols):
                """src [nrows, ncols] -> dst_tile[128, T, RB] logical rows
                [row0, row0+nrows), cols [col0, col0+ncols)."""
                done = 0
                while done < nrows:
                    r = row0 + done
                    t = r // 128
                    p = r % 128
                    n = min(nrows - done, 128 - p)
                    nc.vector.tensor_copy(
                        dst_tile[p:p + n, t, col0:col0 + ncols],
                        src[done:done + n, :])
                    done += n

            # ---------------- xy + index prep ----------------
            xy_sb, em_idx, h_idx, maskf_t, y_sb, lt_sb, feat_sb = \
                [], [], [], [], [], [], []
            for h in range(2):
                t = pst.tile([RH, 50], i32, tag=f"xy{h}")
                nc.sync.dma_start(out=t[:], in_=xy[h * RH:(h + 1) * RH, :])
                xy_sb.append(t)
                aid = t[:, 0:NLEN]
                maskI = pst.tile([RH, NLEN], i32, tag=f"maskI{h}")
                nc.vector.tensor_scalar(maskI[:], aid, PAD, 400000,
                                        op0=ALU.is_equal, op1=ALU.mult)
                emi = pst.tile([RH, NLEN], i32, tag=f"emi{h}")
                nc.vector.tensor_tensor(out=emi[:], in0=aid, in1=maskI[:],
                                        op=ALU.add)
                em_idx.append(emi)
                hi = pst.tile([RH, NLEN, 3], i32, tag=f"hi{h}")
                hr = t[:, NLEN:2 * NLEN]
                nc.vector.tensor_tensor(out=hi[:, :, 0], in0=hr, in1=maskI[:],
                                        op=ALU.add)
                for s in (1, 2):
                    hc = sp.tile([RH, NLEN], i32, tag="hclip")
                    nc.vector.tensor_scalar(hc[:], hr, s, H_CLIP,
                                            op0=ALU.add, op1=ALU.min)
                    nc.vector.tensor_tensor(out=hi[:, :, s], in0=hc[:],
                                            in1=maskI[:], op=ALU.add)
                h_idx.append(hi)
                mf = pst.tile([RH, NLEN], f32, tag=f"maskf{h}")
                nc.vector.tensor_scalar(mf[:], aid, PAD, None,
                                        op0=ALU.not_equal)
                maskf_t.append(mf)
                y_sb.append(t[:, 3 * NLEN:4 * NLEN])
                lt_sb.append(t[:, 4 * NLEN:5 * NLEN])
                feat_sb.append(t[:, 2 * NLEN:3 * NLEN])

            # ---------------- negT: gather + normalize + transpose --------
            negidx_sb = pst.tile([128, NEG_T], i32, tag="negidx")
            nc.sync.dma_start(
                out=negidx_sb[:],
                in_=negidx.ap().rearrange("(t p) k -> p t k", p=128))
            negT = pst.tile([128, 4, NEG_LP], bf16, tag="negT")
            for t in range(NEG_T):
                g = gp.tile([128, EMBP], f32, tag="g512", bufs=2)
                nc.vector.memset(g[:], 0.0)
                nc.gpsimd.indirect_dma_start(
                    out=g[:, :EMB], out_offset=None, in_=emb_w[:],
                    in_offset=bass.IndirectOffsetOnAxis(
                        ap=negidx_sb[:, t:t + 1], axis=0),
                    bounds_check=N_AID - 1, oob_is_err=False)
                ss = sp.tile([128, 1], f32, tag="negss")
                sqs = wp.tile([128, EMB], f32, tag="dummy500", name="dummy",
                              bufs=1)
                nc.scalar.activation(out=sqs[:], in_=g[:, :EMB],
                                     func=AF.Square, accum_out=ss[:, :1])
                rs = sp.tile([128, 1], f32, tag="negrs")
                nc.scalar.activation(out=rs[:], in_=ss[:], func=AF.Sqrt,
                                     bias=constv(1e-12))
                nc.vector.reciprocal(out=rs[:], in_=rs[:])
                gb = gp.tile([128, EMBP], bf16, tag="gb512", bufs=2)
                nc.vector.tensor_scalar(gb[:], g[:], rs[:, :1], None,
                                        op0=ALU.mult)
                for d in range(4):
                    tp = tr_psum()
                    nc.tensor.transpose(out=tp[:],
                                        in_=gb[:, d * 128:(d + 1) * 128],
                                        identity=identb[:])
                    nc.vector.tensor_copy(negT[:, d, t * 128:(t + 1) * 128],
                                          tp[:])

            # ---------------- feats gathers -> featsT ----------------
            featsT = pst.tile([128, INP_T, RB], bf16, tag="mlpbuf_a")
            nc.vector.memset(featsT[:, 44, :], 0.0)
            for h in range(2):
                c0 = h * RH
                oth = gp.tile([RH, 32], bf16, tag="oth")
                nc.vector.memset(oth[:], 0.0)
                featf = sp.tile([RH, NLEN], f32, tag="featf")
                nc.vector.tensor_copy(featf[:], feat_sb[h])
                nc.vector.tensor_tensor(out=featf[:], in0=featf[:],
                                        in1=maskf_t[h][:], op=ALU.mult)
                emh_all = gp.tile([RH, EMBP], bf16, tag="emh_all")
                nc.vector.memset(emh_all[:], 0.0)
                for l in range(NLEN):
                    # em block
                    g = gp.tile([RH, EMBP], f32, tag="g512", bufs=2)
                    nc.vector.memset(g[:], 0.0)
                    nc.gpsimd.indirect_dma_start(
                        out=g[:, :EMB], out_offset=None, in_=emb_w[:],
                        in_offset=bass.IndirectOffsetOnAxis(
                            ap=em_idx[h][:, l:l + 1], axis=0),
                        bounds_check=N_AID - 1, oob_is_err=False)
                    gb = gp.tile([RH, EMBP], bf16, tag="gb512", bufs=2)
                    nc.vector.tensor_copy(gb[:], g[:])
                    for d in range(4):
                        tp = tr_psum((128, RH))
                        nc.tensor.transpose(
                            out=tp[:], in_=gb[:, d * 128:(d + 1) * 128],
                            identity=identb[:RH, :RH])
                        nc.vector.tensor_copy(
                            featsT[:, l * 4 + d, c0:c0 + RH], tp[:, :])
                    # hour gathers (full 52-wide rows; shift 0 has sin/cos)
                    hg = []
                    for s in range(3):
                        gh = gp.tile([RH, 52], f32, tag="hg", bufs=4)
                        nc.vector.memset(gh[:], 0.0)
                        nc.gpsimd.indirect_dma_start(
                            out=gh[:], out_offset=None, in_=hour_tbl[:],
                            in_offset=bass.IndirectOffsetOnAxis(
                                ap=h_idx[h][:, l, s:s + 1], axis=0),
                            bounds_check=H_CLIP, oob_is_err=False)
                        hg.append(gh)
                    nc.vector.tensor_copy(oth[:, 3 * l:3 * l + 1],
                                          hg[0][:, 50:51])
                    nc.vector.tensor_copy(oth[:, 3 * l + 1:3 * l + 2],
                                          hg[0][:, 51:52])
                    nc.vector.tensor_copy(oth[:, 3 * l + 2:3 * l + 3],
                                          featf[:, l:l + 1])
                    tmp = sp.tile([RH, 50], f32, tag="emhtmp")
                    nc.vector.tensor_tensor(out=tmp[:], in0=hg[0][:, :50],
                                            in1=hg[1][:, :50], op=ALU.add)
                    nc.vector.tensor_tensor(
                        out=emh_all[:, l * 50:(l + 1) * 50], in0=tmp[:],
                        in1=hg[2][:, :50], op=ALU.add)
                # emh region: rows [5120, 5632) = featsT tiles 40..43
                for d in range(4):
                    tp = tr_psum((128, RH))
                    nc.tensor.transpose(
                        out=tp[:], in_=emh_all[:, d * 128:(d + 1) * 128],
                        identity=identb[:RH, :RH])
                    nc.vector.tensor_copy(featsT[:, 40 + d, c0:c0 + RH],
                                          tp[:, :])
                # other block: rows [5632, 5664) = tile 44 partitions 0..32
                tp = tr_psum((128, RH))
                nc.tensor.transpose(out=tp[:32, :], in_=oth[:, :32],
                                    identity=identb[:RH, :RH])
                nc.vector.tensor_copy(featsT[:32, 44, c0:c0 + RH], tp[:32, :])

            # ---------------- BN stats + apply ----------------
            dbg_refs = {}

            def bn_stats_apply(x_tile, ntiles, stats_loc, stats_glob,
                               bnp_dram, name):
                st = sp.tile([128, ntiles, 2], f32, tag="st", name="st")
                dbg_refs[name + "_st"] = st
                scr = wp.tile([128, RB], f32, tag="dummy500", name="dummy", bufs=1)
                for t in range(ntiles):
                    nc.scalar.activation(out=scr[:], in_=x_tile[:, t, :],
                                         func=AF.Copy,
                                         accum_out=st[:, t, 0:1])
                    nc.scalar.activation(out=scr[:], in_=x_tile[:, t, :],
                                         func=AF.Square,
                                         accum_out=st[:, t, 1:2])
                nc.sync.dma_start(
                    out=stats_loc.rearrange("(t p) k -> p t k", p=128),
                    in_=st[:])
                nc.gpsimd.collective_compute(
                    "AllReduce", ALU.add, replica_groups=RG,
                    ins=[stats_loc.opt()], outs=[stats_glob.opt()])
                stg = sp.tile([128, ntiles, 2], f32, tag="stg", name="stg")
                dbg_refs[name + "_stg"] = stg
                nc.sync.dma_start(
                    out=stg[:],
                    in_=stats_glob.rearrange("(t p) k -> p t k", p=128))
                bnt = sp.tile([128, ntiles, 2], f32, tag="bnt", name="bnt")
                nc.sync.dma_start(
                    out=bnt[:],
                    in_=bnp_dram.ap().rearrange("(t p) k -> p t k", p=128))
                mean = sp.tile([128, ntiles], f32, tag="mean", name="mean")
                var = sp.tile([128, ntiles], f32, tag="var", name="var")
                nc.vector.tensor_scalar(mean[:], stg[:, :, 0], 1.0 / B, None,
                                        op0=ALU.mult)
                nc.vector.tensor_scalar(var[:], stg[:, :, 1], 1.0 / B, None,
                                        op0=ALU.mult)
                m2 = sp.tile([128, ntiles], f32, tag="m2", name="m2")
                nc.vector.tensor_tensor(out=m2[:], in0=mean[:], in1=mean[:],
                                        op=ALU.mult)
                nc.vector.tensor_tensor(out=var[:], in0=var[:], in1=m2[:],
                                        op=ALU.subtract)
                rstd = sp.tile([128, ntiles], f32, tag="rstd", name="rstd")
                nc.scalar.activation(out=rstd[:], in_=var[:], func=AF.Sqrt,
                                     bias=constv(1e-5))
                nc.vector.reciprocal(out=rstd[:], in_=rstd[:])
                sc = sp.tile([128, ntiles], f32, tag="sc", name="sc")
                nc.vector.tensor_tensor(out=sc[:], in0=rstd[:],
                                        in1=bnt[:, :, 0], op=ALU.mult)
                sh = sp.tile([128, ntiles], f32, tag="sh", name="sh")
                nc.vector.tensor_tensor(out=sh[:], in0=mean[:], in1=sc[:],
                                        op=ALU.mult)
                nc.vector.tensor_tensor(out=sh[:], in0=bnt[:, :, 1], in1=sh[:],
                                        op=ALU.subtract)
                for t in range(ntiles):
                    nc.vector.tensor_scalar(x_tile[:, t, :], x_tile[:, t, :],
                                            sc[:, t:t + 1], sh[:, t:t + 1],
                                            op0=ALU.mult, op1=ALU.add)

            s1_loc = dram.tile([INP_P, 2], f32)
            s1_glob = dram.tile([INP_P, 2], f32)
            s2_loc = dram.tile([H1_P, 2], f32)
            s2_glob = dram.tile([H1_P, 2], f32)
            s3_loc = dram.tile([H2_P, 2], f32)
            s3_glob = dram.tile([H2_P, 2], f32)

            bn_stats_apply(featsT, INP_T, s1_loc, s1_glob, bn1, "s1")
            if DEBUG_OUT:
                nc.sync.dma_start(out=dbg_s1l.rearrange(
                    "(t p) k -> p t k", p=128), in_=dbg_refs["s1_st"][:])
                nc.sync.dma_start(out=dbg_s1g.rearrange(
                    "(t p) k -> p t k", p=128), in_=dbg_refs["s1_stg"][:])
                f0c = sp.tile([128, RB], f32, tag="dbgbuf", name="dbgbuf")
                nc.vector.tensor_copy(f0c[:], featsT[:, 0, :])
                nc.sync.dma_start(out=dbg_f0[:], in_=f0c[:])
                f44c = sp.tile([128, RB], f32, tag="dbgbuf", name="dbgbuf")
                nc.vector.tensor_copy(f44c[:], featsT[:, 44, :])
                nc.sync.dma_start(out=dbg_f44[:], in_=f44c[:])

            # ---------------- MLP layers ----------------
            def mlp_layer(x_tile, in_tiles, w_dram, b_dram, out_tiles, name,
                          htag, gelu=True):
                h_tile = pst.tile([128, out_tiles, RB], bf16, tag=htag,
                                  name=f"h_{name}")
                bt = sp.tile([128, out_tiles], f32, tag=f"b_{name}")
                nc.sync.dma_start(
                    out=bt[:],
                    in_=b_dram.ap().rearrange("(t p) k -> p t k", p=128))
                GW = 2
                for og in range(0, out_tiles, GW):
                    on = min(GW, out_tiles - og)
                    psums = [mmps.tile([128, RB], f32, space="PSUM", tag="mmp",
                                       name="mmp") for _ in range(on)]
                    for it in range(in_tiles):
                        wsl = wp.tile([128, on, 128], bf16, tag="wsl", name="wsl")
                        nc.sync.dma_start(
                            out=wsl[:, :on, :],
                            in_=w_dram[it * 128:(it + 1) * 128,
                                       og * 128:(og + on) * 128]
                            .rearrange("p (a b) -> p a b", a=on))
                        for o in range(on):
                            nc.tensor.matmul(
                                psums[o][:], lhsT=wsl[:, o, :],
                                rhs=x_tile[:, it, :],
                                start=(it == 0), stop=(it == in_tiles - 1))
                    for o in range(on):
                        nc.scalar.activation(
                            out=h_tile[:, og + o, :], in_=psums[o][:],
                            func=AF.Gelu if gelu else AF.Identity,
                            bias=bt[:, og + o:og + o + 1], scale=1.0)
                return h_tile

            h1 = mlp_layer(featsT, INP_T, w1p, b1p, H1_T, "h1", "mlpbuf_b")
            if DEBUG_OUT:
                h1c = sp.tile([128, RB], f32, tag="dbgbuf", name="dbgbuf")
                nc.vector.tensor_copy(h1c[:], h1[:, 0, :])
                nc.sync.dma_start(out=dbg_h1[:], in_=h1c[:])
            bn_stats_apply(h1, H1_T, s2_loc, s2_glob, bn2, "s2")
            h2 = mlp_layer(h1, H1_T, w2p, b2p, H2_T, "h2", "mlpbuf_a")
            bn_stats_apply(h2, H2_T, s3_loc, s3_glob, bn3, "s3")
            qT = mlp_layer(h2, H2_T, w3p, b3p, 4, "q", "mlpbuf_b", gelu=False)

            # ---------------- q -> row layout ----------------
            q_row = []
            for h in range(2):
                qr = pst.tile([RH, EMBP], f32, tag=f"qrow{h}")
                for d in range(4):
                    tp = tr_psum((128, 128))
                    nc.tensor.transpose(
                        out=tp[:RH, :], in_=qT[:, d, h * RH:(h + 1) * RH],
                        identity=identb[:])
                    nc.vector.tensor_copy(qr[:, d * 128:(d + 1) * 128],
                                          tp[:RH, :])
                q_row.append(qr)
            if DEBUG_OUT:
                for h in range(2):
                    nc.sync.dma_start(out=dbg_q[h * RH:(h + 1) * RH, :],
                                      in_=q_row[h][:])

            # ---------------- x_em / y_em / c_pos / x0_hat ----------------
            cpos_loc = dram.tile([RB, 1], f32)
            cpos_all = dram.tile([B, 1], f32)
            x0_loc = dram.tile([RB, EMBP], bf16)
            x0_all = dram.tile([B, EMBP], bf16)
            for h in range(2):
                cp = sp.tile([RH, NLAB], f32, tag="cp")
                for l in range(NLAB):
                    ltv = gp.tile([RH, EMB], f32, tag="ltv")
                    nc.gpsimd.indirect_dma_start(
                        out=ltv[:], out_offset=None, in_=lt_w[:],
                        in_offset=bass.IndirectOffsetOnAxis(
                            ap=lt_sb[h][:, l:l + 1], axis=0))
                    xe = gp.tile([RH, EMB], f32, tag="xe")
                    nc.vector.tensor_tensor(out=xe[:], in0=q_row[h][:, :EMB],
                                            in1=ltv[:], op=ALU.add)
                    ye = gp.tile([RH, EMB], f32, tag="ye")
                    nc.gpsimd.indirect_dma_start(
                        out=ye[:], out_offset=None, in_=emb_w[:],
                        in_offset=bass.IndirectOffsetOnAxis(
                            ap=y_sb[h][:, l:l + 1], axis=0))
                    ssx = sp.tile([RH, 1], f32, tag="ssx")
                    ssy = sp.tile([RH, 1], f32, tag="ssy")
                    scr = wp.tile([RH, EMB], f32, tag="dummy500",
                                  name="dummy", bufs=1)
                    nc.scalar.activation(out=scr[:], in_=xe[:], func=AF.Square,
                                         accum_out=ssx[:, :1])
                    scr2 = wp.tile([RH, EMB], f32, tag="dummy500",
                                   name="dummy", bufs=1)
                    nc.scalar.activation(out=scr2[:], in_=ye[:],
                                         func=AF.Square, accum_out=ssy[:, :1])
                    prod = sp.tile([RH, 1], f32, tag="prodn")
                    nc.vector.tensor_tensor(out=prod[:], in0=ssx[:],
                                            in1=ssy[:], op=ALU.mult)
                    rsxy = sp.tile([RH, 1], f32, tag="rsxy")
                    nc.scalar.activation(out=rsxy[:], in_=prod[:],
                                         func=AF.Sqrt, bias=constv(1e-24, RH))
                    nc.vector.reciprocal(out=rsxy[:], in_=rsxy[:])
                    dotv = sp.tile([RH, 1], f32, tag="dotv")
                    mscr = wp.tile([RH, EMB], f32, tag="mscr", bufs=1)
                    nc.vector.tensor_tensor(out=mscr[:], in0=xe[:], in1=ye[:],
                                            op=ALU.mult)
                    dscr = wp.tile([RH, EMB], f32, tag="dummy500",
                                   name="dummy", bufs=1)
                    nc.scalar.activation(out=dscr[:], in_=mscr[:],
                                         func=AF.Copy, accum_out=dotv[:, :1])
                    nc.vector.tensor_tensor(out=cp[:, l:l + 1], in0=dotv[:],
                                            in1=rsxy[:], op=ALU.mult)
                    if l == 0:
                        rsx = sp.tile([RH, 1], f32, tag="rsx")
                        nc.scalar.activation(out=rsx[:], in_=ssx[:],
                                             func=AF.Sqrt, bias=constv(1e-24, RH))
                        nc.vector.reciprocal(out=rsx[:], in_=rsx[:])
                        x0h = gp.tile([RH, EMBP], bf16, tag="x0h")
                        nc.vector.memset(x0h[:], 0.0)
                        nc.vector.tensor_scalar(x0h[:, :EMB], xe[:],
                                                rsx[:, :1], None, op0=ALU.mult)
                        nc.sync.dma_start(out=x0_loc[h * RH:(h + 1) * RH, :],
                                          in_=x0h[:])
                ym = sp.tile([RH, NLAB], f32, tag="ym")
                nc.vector.tensor_scalar(ym[:], y_sb[h][:, :], PAD, None,
                                        op0=ALU.is_equal)
                cpm = sp.tile([RH, NLAB], f32, tag="cpm")
                nc.vector.tensor_tensor(out=cpm[:], in0=cp[:], in1=ym[:],
                                        op=ALU.add)
                cmin = sp.tile([RH, 1], f32, tag="cmin")
                nc.vector.tensor_reduce(out=cmin[:, :1], in_=cpm[:],
                                        axis=mybir.AxisListType.X, op=ALU.min)
                ltf = sp.tile([RH, NLAB], f32, tag="ltf")
                nc.vector.tensor_copy(ltf[:], lt_sb[h][:, :])
                cl = sp.tile([RH, NLAB], f32, tag="cl")
                nc.vector.tensor_tensor(out=cl[:], in0=cp[:], in1=ltf[:],
                                        op=ALU.mult)
                num = sp.tile([RH, 1], f32, tag="num")
                den = sp.tile([RH, 1], f32, tag="den")
                nc.vector.tensor_reduce(out=num[:, :1], in_=cl[:],
                                        axis=mybir.AxisListType.X, op=ALU.add)
                nc.vector.tensor_reduce(out=den[:, :1], in_=ltf[:],
                                        axis=mybir.AxisListType.X, op=ALU.add)
                nc.vector.reciprocal(out=den[:], in_=den[:])
                cmean = sp.tile([RH, 1], f32, tag="cmean")
                nc.vector.tensor_tensor(out=cmean[:], in0=num[:], in1=den[:],
                                        op=ALU.mult)
                cpos = sp.tile([RH, 1], f32, tag="cpos")
                nc.vector.tensor_tensor(out=cpos[:], in0=cmean[:],
                                        in1=cmin[:], op=ALU.add)
                nc.vector.tensor_scalar_mul(cpos[:], cpos[:], 0.5)
                nc.sync.dma_start(out=cpos_loc[h * RH:(h + 1) * RH, :],
                                  in_=cpos[:])

            nc.gpsimd.collective_compute(
                "AllGather", ALU.bypass, replica_groups=RG,
                ins=[cpos_loc.opt()], outs=[cpos_all.opt()])
            nc.gpsimd.collective_compute(
                "AllGather", ALU.bypass, replica_groups=RG,
                ins=[x0_loc.opt()], outs=[x0_all.opt()])

            # ---------------- xT [128, 4, 2000] ----------------
            xT = pst.tile([128, 4, B], bf16, tag="xT")
            for g in range(RT):
                xrow = gp.tile([RH, EMBP], bf16, tag="xrow")
                nc.sync.dma_start(out=xrow[:],
                                  in_=x0_all[g * RH:(g + 1) * RH, :])
                for d in range(4):
                    tp = tr_psum((128, RH))
                    nc.tensor.transpose(out=tp[:],
                                        in_=xrow[:, d * 128:(d + 1) * 128],
                                        identity=identb[:RH, :RH])
                    nc.vector.tensor_copy(xT[:, d, g * RH:(g + 1) * RH],
                                          tp[:, :])

            # ---------------- sims + threshold machinery ----------------
            sims_dram = dram.tile([B, NEG_LP], bf16)
            cnt0_loc = dram.tile([B, 1], f32)
            cnt0_glob = dram.tile([B, 1], f32)
            cnt1_loc = dram.tile([B, 1], f32)
            cnt1_glob = dram.tile([B, 1], f32)
            m_loc = dram.tile([B, 1], f32)
            m_glob = dram.tile([B, 1], f32)
            a_loc = dram.tile([B, 1], f32)
            a_glob = dram.tile([B, 1], f32)

            sgn0 = pst.tile([RH, RT], f32, tag="sgn0")
            mloc_t = pst.tile([RH, RT], f32, tag="mloc")
            NJ = (NEG_LP + 511) // 512  # 15
            for g in range(RT):
                sims_sb = wp.tile([RH, NEG_LP], bf16, tag="sims", bufs=2)
                for jg in range(0, NJ, 4):
                    jn = min(4, NJ - jg)
                    psums = [mmps.tile([RH, 512], f32, space="PSUM", tag="mmp",
                                       name="mmp") for _ in range(jn)]
                    for d in range(4):
                        for j in range(jn):
                            w = min(512, NEG_LP - (jg + j) * 512)
                            nc.tensor.matmul(
                                psums[j][:, :w],
                                lhsT=xT[:, d, g * RH:(g + 1) * RH],
                                rhs=negT[:, d,
                                         (jg + j) * 512:(jg + j) * 512 + w],
                                start=(d == 0), stop=(d == 3))
                    for j in range(jn):
                        w = min(512, NEG_LP - (jg + j) * 512)
                        nc.vector.tensor_copy(
                            sims_sb[:, (jg + j) * 512:(jg + j) * 512 + w],
                            psums[j][:, :w])
                nc.sync.dma_start(out=sims_dram[g * RH:(g + 1) * RH, :],
                                  in_=sims_sb[:])
                nc.vector.tensor_reduce(out=mloc_t[:, g:g + 1],
                                        in_=sims_sb[:],
                                        axis=mybir.AxisListType.X, op=ALU.max)
                nc.scalar.activation(out=sims_sb[:], in_=sims_sb[:],
                                     func=AF.Sign, bias=constv(-T0, RH),
                                     accum_out=sgn0[:, g:g + 1])

            nc.sync.dma_start(
                out=cnt0_loc.rearrange("(g p) k -> p g k", p=RH),
                in_=sgn0[:])
            nc.sync.dma_start(
                out=m_loc.rearrange("(g p) k -> p g k", p=RH),
                in_=mloc_t[:])
            nc.gpsimd.collective_compute(
                "AllReduce", ALU.add, replica_groups=RG,
                ins=[cnt0_loc.opt()], outs=[cnt0_glob.opt()])
            nc.gpsimd.collective_compute(
                "AllReduce", ALU.max, replica_groups=RG,
                ins=[m_loc.opt()], outs=[m_glob.opt()])

            # t1 = T0 + (c0 - 1000)/NSLOPE with c0 = (sgnsum + N_ALL)/2
            sgn_g = sp.tile([RH, RT], f32, tag="sgn_g")
            nc.sync.dma_start(
                out=sgn_g[:],
                in_=cnt0_glob.rearrange("(g p) k -> p g k", p=RH))
            c0g = sp.tile([RH, RT], f32, tag="c0g")
            nc.vector.tensor_scalar(c0g[:], sgn_g[:], 0.5, N_ALL * 0.5,
                                    op0=ALU.mult, op1=ALU.add)
            if DEBUG_OUT:
                nc.sync.dma_start(
                    out=dbg_cnt.ap().rearrange("(g p) k -> p g k", p=RH),
                    in_=c0g[:])
            t1 = pst.tile([RH, RT], f32, tag="t1")
            nc.vector.tensor_scalar(t1[:], c0g[:], 1.0 / NSLOPE,
                                    float(T0 - 1000.0 / NSLOPE),
                                    op0=ALU.mult, op1=ALU.add)

            # second Newton iteration: count at t1
            sgn1 = pst.tile([RH, RT], f32, tag="sgn1")
            negt1 = pst.tile([RH, RT], f32, tag="negt1")
            nc.vector.tensor_scalar(negt1[:], t1[:], -1.0, None, op0=ALU.mult)
            for g in range(RT):
                sims_sb = wp.tile([RH, NEG_LP], bf16, tag="sims",
                                  name="sims", bufs=2)
                nc.sync.dma_start(out=sims_sb[:],
                                  in_=sims_dram[g * RH:(g + 1) * RH, :])
                nc.scalar.activation(out=sims_sb[:], in_=sims_sb[:],
                                     func=AF.Sign, bias=negt1[:, g:g + 1],
                                     accum_out=sgn1[:, g:g + 1])
            nc.sync.dma_start(
                out=cnt1_loc.rearrange("(g p) k -> p g k", p=RH),
                in_=sgn1[:])
            nc.gpsimd.collective_compute(
                "AllReduce", ALU.add, replica_groups=RG,
                ins=[cnt1_loc.opt()], outs=[cnt1_glob.opt()])
            sgn1g = sp.tile([RH, RT], f32, tag="sgn1g")
            nc.sync.dma_start(
                out=sgn1g[:],
                in_=cnt1_glob.rearrange("(g p) k -> p g k", p=RH))
            c1g = sp.tile([RH, RT], f32, tag="c1g")
            nc.vector.tensor_scalar(c1g[:], sgn1g[:], 0.5, N_ALL * 0.5,
                                    op0=ALU.mult, op1=ALU.add)
            if DEBUG_OUT:
                nc.sync.dma_start(
                    out=dbg_cnt1.ap().rearrange("(g p) k -> p g k", p=RH),
                    in_=c1g[:])
            # slope at t1: NSLOPE * exp(-(t1^2 - T0^2)/(2 sigma^2))
            u = sp.tile([RH, RT], f32, tag="u")
            nc.vector.tensor_tensor(out=u[:], in0=t1[:], in1=t1[:],
                                    op=ALU.mult)
            slope = sp.tile([RH, RT], f32, tag="slope")
            nc.scalar.activation(out=slope[:], in_=u[:], func=AF.Exp,
                                 scale=float(-1.0 / (2 * SIGMA ** 2)),
                                 bias=constv(T0 ** 2 / (2 * SIGMA ** 2), RH))
            nc.vector.tensor_scalar_mul(slope[:], slope[:], NSLOPE)
            nc.vector.reciprocal(out=slope[:], in_=slope[:])
            dc = sp.tile([RH, RT], f32, tag="dc")
            nc.vector.tensor_scalar(dc[:], c1g[:], -1000.0, None, op0=ALU.add)
            nc.vector.tensor_tensor(out=dc[:], in0=dc[:], in1=slope[:],
                                    op=ALU.mult)
            t2 = pst.tile([RH, RT], f32, tag="t2")
            nc.vector.tensor_tensor(out=t2[:], in0=t1[:], in1=dc[:],
                                    op=ALU.add)
            if DEBUG_OUT:
                nc.sync.dma_start(
                    out=dbg_t2.ap().rearrange("(g p) k -> p g k", p=RH),
                    in_=t2[:])

            # final pass: A = sum_j exp(20 relu(x_j - t2))
            n20t2 = pst.tile([RH, RT], f32, tag="n20t2")
            nc.vector.tensor_scalar(n20t2[:], t2[:], -SC, None, op0=ALU.mult)
            a_t = pst.tile([RH, RT], f32, tag="a_t")
            for g in range(RT):
                sims_sb = wp.tile([RH, NEG_LP], bf16, tag="sims",
                                  name="sims", bufs=2)
                nc.sync.dma_start(out=sims_sb[:],
                                  in_=sims_dram[g * RH:(g + 1) * RH, :])
                nc.scalar.activation(out=sims_sb[:], in_=sims_sb[:],
                                     func=AF.Relu, scale=SC,
                                     bias=n20t2[:, g:g + 1])
                nc.scalar.activation(out=sims_sb[:], in_=sims_sb[:],
                                     func=AF.Exp,
                                     accum_out=a_t[:, g:g + 1])
            nc.sync.dma_start(
                out=a_loc.rearrange("(g p) k -> p g k", p=RH),
                in_=a_t[:])
            nc.gpsimd.collective_compute(
                "AllReduce", ALU.add, replica_groups=RG,
                ins=[a_loc.opt()], outs=[a_glob.opt()])

            # per-row loss/acc
            a_g = sp.tile([RH, RT], f32, tag="a_g")
            nc.sync.dma_start(
                out=a_g[:],
                in_=a_glob.rearrange("(g p) k -> p g k", p=RH))
            m_g = sp.tile([RH, RT], f32, tag="m_g")
            nc.sync.dma_start(
                out=m_g[:],
                in_=m_glob.rearrange("(g p) k -> p g k", p=RH))
            cpos_t = sp.tile([RH, RT], f32, tag="cpos_t")
            nc.sync.dma_start(
                out=cpos_t[:],
                in_=cpos_all.rearrange("(g p) k -> p g k", p=RH))
            if DEBUG_OUT:
                nc.sync.dma_start(
                    out=dbg_A.ap().rearrange("(g p) k -> p g k", p=RH),
                    in_=a_g[:])
                nc.sync.dma_start(
                    out=dbg_M.ap().rearrange("(g p) k -> p g k", p=RH),
                    in_=m_g[:])
                nc.sync.dma_start(
                    out=dbg_cpos.ap().rearrange("(g p) k -> p g k", p=RH),
                    in_=cpos_t[:])
            sfree = sp.tile([RH, RT], f32, tag="sfree")
            nc.vector.tensor_scalar(sfree[:], a_g[:], float(1000.0 - N_ALL),
                                    None, op0=ALU.add)
            e20 = sp.tile([RH, RT], f32, tag="e20")
            for g in range(RT):
                nc.scalar.activation(out=e20[:, g:g + 1],
                                     in_=cpos_t[:, g:g + 1], func=AF.Exp,
                                     scale=SC, bias=n20t2[:, g:g + 1])
            inner = sp.tile([RH, RT], f32, tag="inner")
            nc.vector.tensor_tensor(out=inner[:], in0=e20[:], in1=sfree[:],
                                    op=ALU.add)
            lse = sp.tile([RH, RT], f32, tag="lse")
            nc.scalar.activation(out=lse[:], in_=inner[:], func=AF.Ln)
            t2s = sp.tile([RH, RT], f32, tag="t2s")
            nc.vector.tensor_scalar(t2s[:], t2[:], SC, None, op0=ALU.mult)
            nc.vector.tensor_tensor(out=lse[:], in0=lse[:], in1=t2s[:],
                                    op=ALU.add)
            cps = sp.tile([RH, RT], f32, tag="cps")
            nc.vector.tensor_scalar(cps[:], cpos_t[:], SC, None, op0=ALU.mult)
            loss_t = sp.tile([RH, RT], f32, tag="loss_t")
            nc.vector.tensor_tensor(out=loss_t[:], in0=lse[:], in1=cps[:],
                                    op=ALU.subtract)
            acc_t = sp.tile([RH, RT], f32, tag="acc_t")
            nc.vector.tensor_tensor(out=acc_t[:], in0=cpos_t[:], in1=m_g[:],
                                    op=ALU.is_ge)
            nc.sync.dma_start(
                out=out[0:1, :].rearrange("k (g p) -> p g k", p=RH),
                in_=loss_t[:])
            nc.sync.dma_start(
                out=out[1:2, :].rearrange("k (g p) -> p g k", p=RH),
                in_=acc_t[:])

    nc.compile()
    return nc


def host_prep(inputs):
    xy = np.ascontiguousarray(np.asarray(inputs["xy_batch"], dtype=np.int32))
    neg = np.asarray(inputs["neg_aids"], dtype=np.int32)
    emb_w = np.ascontiguousarray(np.asarray(inputs["emb_w"], np.float32))
    emb_h_w = np.asarray(inputs["emb_h_w"], np.float32)
    lt_wv = np.ascontiguousarray(np.asarray(inputs["lt_w"], np.float32))

    hidx = np.arange(H_ROWS)
    ang = (hidx % 24).astype(np.float32) / 24.0 * (2.0 * np.pi)
    hour_tbl = np.ascontiguousarray(np.concatenate(
        [emb_h_w, np.sin(ang)[:, None].astype(np.float32),
         np.cos(ang)[:, None].astype(np.float32)], axis=1))

    old_idx = np.zeros(INP_P, dtype=np.int64)
    valid = np.zeros(INP_P, dtype=bool)
    for l in range(NLEN):
        old_idx[l * 512:l * 512 + EMB] = l * 553 + np.arange(EMB)
        valid[l * 512:l * 512 + EMB] = True
        old_idx[EMH0 + l * 50:EMH0 + (l + 1) * 50] = \
            l * 553 + 503 + np.arange(50)
        valid[EMH0 + l * 50:EMH0 + (l + 1) * 50] = True
        old_idx[OTH0 + l * 3:OTH0 + l * 3 + 3] = l * 553 + EMB + np.arange(3)
        valid[OTH0 + l * 3:OTH0 + l * 3 + 3] = True

    w1 = np.asarray(inputs["w1"], np.float32)
    w2 = np.asarray(inputs["w2"], np.float32)
    w3 = np.asarray(inputs["w3"], np.float32)
    w1p = np.zeros((INP_P, H1_P), np.float32)
    w1p[valid, :w1.shape[1]] = w1[old_idx[valid]]
    w1p = w1p.astype(ml_dtypes.bfloat16)
    w2p = np.zeros((H1_P, H2_P), np.float32)
    w2p[:w2.shape[0], :w2.shape[1]] = w2
    w2p = w2p.astype(ml_dtypes.bfloat16)
    w3p = np.zeros((H2_P, EMBP), np.float32)
    w3p[:w3.shape[0], :w3.shape[1]] = w3
    w3p = w3p.astype(ml_dtypes.bfloat16)

    b1p = np.zeros((H1_P, 1), np.float32)
    b1p[:5530, 0] = np.asarray(inputs["b1"], np.float32)
    b2p = np.zeros((H2_P, 1), np.float32)
    b2p[:2765, 0] = np.asarray(inputs["b2"], np.float32)
    b3p = np.zeros((EMBP, 1), np.float32)
    b3p[:EMB, 0] = np.asarray(inputs["b3"], np.float32)

    bn1 = np.zeros((INP_P, 2), np.float32)
    bn1[valid, 0] = np.asarray(inputs["bn1_g"], np.float32)[old_idx[valid]]
    bn1[valid, 1] = np.asarray(inputs["bn1_b"], np.float32)[old_idx[valid]]
    bn2 = np.zeros((H1_P, 2), np.float32)
    bn2[:5530, 0] = np.asarray(inputs["bn2_g"], np.float32)
    bn2[:5530, 1] = np.asarray(inputs["bn2_b"], np.float32)
    bn3 = np.zeros((H2_P, 2), np.float32)
    bn3[:2765, 0] = np.asarray(inputs["bn3_g"], np.float32)
    bn3[:2765, 1] = np.asarray(inputs["bn3_b"], np.float32)

    in_maps = []
    for c in range(N_CORES):
        nidx = np.full((NEG_LP, 1), 400000, np.int32)
        nidx[:NEG_L, 0] = neg[c * NEG_L:(c + 1) * NEG_L]
        in_maps.append(dict(
            xy=np.ascontiguousarray(xy[c * RB:(c + 1) * RB]),
            negidx=nidx, emb_w=emb_w, hour_tbl=hour_tbl, lt_w=lt_wv,
            w1p=w1p, w2p=w2p, w3p=w3p, b1p=b1p, b2p=b2p, b3p=b3p,
            bn1=bn1, bn2=bn2, bn3=bn3))
    return in_maps


def get_nc():
    if "nc" not in _NC_CACHE:
        _NC_CACHE["nc"] = build_nc()
    return _NC_CACHE["nc"]


def run(inputs, trace=False):
    nc = get_nc()
    in_maps = host_prep(inputs)
    return run_bass_kernel_spmd(nc, in_maps, list(range(N_CORES)),
                                trace=trace)


def kernel(**inputs):
    res = run(inputs)
    o = res.results[0]["out"]
    return np.float32(o[0].mean()), np.float32(o[1].mean())


if __name__ == "__main__":
    print("building...")
    get_nc()
    print("built ok")
